# revision 4
# baseline (speedup 1.0000x reference)
"""BitLinear (input-RMSNorm + ternary-quantized linear) on 8 TRN2 NeuronCores.

Math (reference):
  xn    = x * rsqrt(mean(x^2, -1) + eps) * g
  w     = weight * rsqrt(mean(weight^2, 1) + eps)          (row RMS norm)
  am    = mean(|w|, 1)
  w_q   = sign(w) * (|w| > 0.5*am)                          (ternary)
  out   = xn @ (w_q * am * row_scale).T + bias

Kernel strategy (per core, data-parallel over B*S rows; weight replicated):
  - The row rsqrt of x commutes with the matmul: apply it to the OUTPUT
    (per-partition scalar).  g is applied to x^T right after the on-chip
    transpose (per-partition in transposed layout).  alpha = am*row_scale
    is applied in the epilogue via a broadcast row (per-free), bias too.
  - The quantized weight is computed on chip as PURE ternary {-1,0,+1}.
    |w| > 0.5*mean|w| is evaluated in the raw-weight domain (the rsqrt
    factor cancels); for the fixed benchmark data the smallest relative
    margin to the threshold is 5.4e-7, far above the ~2e-7 rounding
    differences vs the reference, so no mask flips.
  - Matmul runs on the PE in float32r (11-bit mantissa, 1 cycle/row vs 4
    for fp32).  x is split exactly as x = hi + lo with hi = rnd_f32r(x):
    both halves are f32r-exact, products with ternary weights are exact,
    PSUM accumulates in fp32 -> full fp32 accuracy at half the fp32 cost.
    (SPLIT=False drops the lo pass: 2x faster matmul, ~1e-4 rel err.)
"""

import sys

if "/opt/trn_rl_repo" not in sys.path:
    sys.path.insert(0, "/opt/trn_rl_repo")

from contextlib import ExitStack

import numpy as np

import concourse.bass as bass
import concourse.mybir as mybir
import concourse.tile as tile
from concourse import bacc, bass_utils
from concourse.masks import make_identity

B, S, DIN, DOUT = 4, 4096, 2048, 2048
NCORES = 8
SC = B * S // NCORES      # 2048 rows of x per core
P = 128
KT = DIN // P             # 16 k-tiles
ST = SC // P              # 16 s-tiles per core
CH = 512                  # psum chunk (one bank of fp32)
NCH = DOUT // CH          # 4 chunks
EPS = 1e-8
SPLIT = True              # exact hi/lo split (False: single f32r pass)

f32 = mybir.dt.float32
f32r = mybir.dt.float32r
AF = mybir.ActivationFunctionType
OP = mybir.AluOpType
AX = mybir.AxisListType


def build_module(split=SPLIT):
    nc = bacc.Bacc("TRN2", target_bir_lowering=False)
    x_d = nc.declare_dram_parameter("x", [SC, DIN], f32, isOutput=False)
    w_d = nc.declare_dram_parameter("weight", [DOUT, DIN], f32, isOutput=False)
    rs_d = nc.declare_dram_parameter("row_scale", [DOUT, 1], f32, isOutput=False)
    b_d = nc.declare_dram_parameter("bias", [DOUT], f32, isOutput=False)
    g_d = nc.declare_dram_parameter("g", [DIN], f32, isOutput=False)
    o_d = nc.declare_dram_parameter("out", [SC, DOUT], f32, isOutput=True)

    with tile.TileContext(nc) as tc, ExitStack() as ctx:
        const = ctx.enter_context(tc.tile_pool(name="const", bufs=1))
        dramp = ctx.enter_context(tc.tile_pool(name="dramp", bufs=1, space="DRAM"))
        big = ctx.enter_context(tc.tile_pool(name="big", bufs=2))
        hip = ctx.enter_context(tc.tile_pool(name="hip", bufs=2))
        lop = ctx.enter_context(tc.tile_pool(name="lop", bufs=2))
        outp = ctx.enter_context(tc.tile_pool(name="outp", bufs=4))
        smp = ctx.enter_context(tc.tile_pool(name="smp", bufs=2))
        pmm = ctx.enter_context(tc.tile_pool(name="pmm", bufs=6, space="PSUM"))
        ptp = ctx.enter_context(tc.tile_pool(name="ptp", bufs=2, space="PSUM"))

        # ---- constants ----
        w2 = const.tile([P, KT, DOUT], f32r)       # ternary weight, [i, o] layout
        bias_b = const.tile([P, DOUT], f32)        # bias broadcast to all partitions
        nalpha_b = const.tile([P, DOUT], f32)      # -alpha broadcast
        ident32 = const.tile([P, P], f32)
        ident32r = const.tile([P, P], f32r)
        make_identity(nc, ident32)
        nc.vector.tensor_copy(ident32r, ident32)
        eps_t = const.tile([P, 1], f32)
        nc.vector.memset(eps_t, EPS)
        g_sb = const.tile([P, KT], f32)            # g[i], i = k*128 + p  -> [p, k]
        nc.gpsimd.dma_start(out=g_sb, in_=g_d.rearrange("(k p) -> p k", p=P))
        rs_sb = const.tile([P, KT], f32)           # row_scale[o], o = j*128+p
        nc.gpsimd.dma_start(
            out=rs_sb, in_=rs_d.rearrange("(j p) one -> p (j one)", p=P)
        )
        # per-w-tile stats, column j = o-tile j
        sabs = const.tile([P, KT], f32)
        rw = const.tile([P, KT], f32)
        traw = const.tile([P, KT], f32)
        nalpha_c = const.tile([P, KT], f32)
        nalpha_scr = dramp.tile([DOUT], f32)

        # bias broadcast: DRAM [DOUT] replicated over 128 partitions
        bias_ap = b_d[:]
        nc.gpsimd.dma_start(
            out=bias_b,
            in_=bass.AP(
                tensor=bias_ap.tensor, offset=bias_ap.offset,
                ap=[[0, P]] + list(bias_ap.ap),
            ),
        )

        # ---- weight prep: stats -> ternary -> transpose into w2 ----
        for j in range(KT):
            w_t = big.tile([P, DIN], f32, name="xw")
            nc.sync.dma_start(out=w_t, in_=w_d[j * P : (j + 1) * P, :])
            scr4 = smp.tile([P, 4], f32, name="scr4")

            # ss = sum(w^2) over free dim, chunked through outp scratch
            for c in range(4):
                dump = outp.tile([P, CH], f32, name="ob")
                nc.scalar.activation(
                    dump, w_t[:, c * CH : (c + 1) * CH], AF.Square,
                    accum_out=scr4[:, c : c + 1],
                )
            nc.vector.tensor_tensor(
                scr4[:, 0:1], scr4[:, 0:1], scr4[:, 1:2], op=OP.add
            )
            nc.vector.tensor_tensor(
                scr4[:, 2:3], scr4[:, 2:3], scr4[:, 3:4], op=OP.add
            )
            nc.vector.tensor_tensor(
                scr4[:, 0:1], scr4[:, 0:1], scr4[:, 2:3], op=OP.add
            )
            # rw_j = sqrt(ss/DIN + eps) ; then reciprocal in place
            nc.scalar.activation(
                rw[:, j : j + 1], scr4[:, 0:1], AF.Sqrt,
                bias=eps_t, scale=1.0 / DIN,
            )
            nc.vector.reciprocal(rw[:, j : j + 1], rw[:, j : j + 1])
            # sumabs = sum(|w|)
            nc.vector.tensor_reduce(
                sabs[:, j : j + 1], w_t, axis=AX.X, op=OP.add,
                apply_absolute_value=True,
            )
            # threshold in the raw-weight domain: traw = 0.5*mean|w|
            nc.vector.tensor_scalar(
                traw[:, j : j + 1], sabs[:, j : j + 1], 0.5 / DIN, None, op0=OP.mult
            )
            # -alpha = ((-mean|w|) * r) * row_scale
            nc.vector.tensor_scalar(
                scr4[:, 1:2], sabs[:, j : j + 1], -1.0 / DIN, None, op0=OP.mult
            )
            nc.vector.tensor_tensor(
                scr4[:, 1:2], scr4[:, 1:2], rw[:, j : j + 1], op=OP.mult
            )
            nc.vector.tensor_tensor(
                nalpha_c[:, j : j + 1], scr4[:, 1:2], rs_sb[:, j : j + 1],
                op=OP.mult,
            )
            # ntraw into scr4[:,3]
            nc.vector.tensor_scalar(
                scr4[:, 3:4], sabs[:, j : j + 1], -0.5 / DIN, None, op0=OP.mult
            )
            # ternary (negated): e = (w < -traw) - (w > traw)  in {-1,0,+1}
            a_t = lop.tile([P, DIN], f32, name="lo")
            nc.gpsimd.tensor_scalar(
                a_t, w_t, traw[:, j : j + 1], None, op0=OP.is_gt
            )
            e_t = hip.tile([P, DIN], f32r, name="hi")
            nc.vector.scalar_tensor_tensor(
                out=e_t, in0=w_t, scalar=scr4[:, 3:4], in1=a_t,
                op0=OP.is_lt, op1=OP.subtract,
            )
            # transpose e into w2[:, :, j*128:(j+1)*128]
            for kk in range(KT // 4):
                pt = ptp.tile([P, 4, P], f32r, name="pt")
                for q in range(4):
                    k = kk * 4 + q
                    nc.tensor.transpose(
                        pt[:, q, :], e_t[:, k * P : (k + 1) * P], ident32r
                    )
                nc.scalar.activation(
                    w2[:, kk * 4 : kk * 4 + 4, j * P : (j + 1) * P], pt, AF.Copy
                )

        # gather -alpha into o-major DRAM scratch, then broadcast to nalpha_b
        nc.gpsimd.dma_start(
            out=nalpha_scr.rearrange("(j p) -> p j", p=P), in_=nalpha_c
        )
        nc.gpsimd.dma_start(
            out=nalpha_b,
            in_=bass.AP(
                tensor=nalpha_scr.tensor, offset=nalpha_scr.offset,
                ap=[[0, P]] + list(nalpha_scr.ap),
            ),
        )

        # ---- main loop over s-tiles ----
        for t in range(ST):
            x_t = big.tile([P, DIN], f32, name="xw")
            nc.sync.dma_start(out=x_t, in_=x_d[t * P : (t + 1) * P, :])
            scr4 = smp.tile([P, 4], f32, name="scr4")

            # ssx = sum(x^2) -> sclx = 1/sqrt(ssx/DIN + eps)
            for c in range(4):
                dump = outp.tile([P, CH], f32, name="ob")
                nc.scalar.activation(
                    dump, x_t[:, c * CH : (c + 1) * CH], AF.Square,
                    accum_out=scr4[:, c : c + 1],
                )
            sclx = smp.tile([P, 1], f32)
            nc.vector.tensor_tensor(
                scr4[:, 0:1], scr4[:, 0:1], scr4[:, 1:2], op=OP.add
            )
            nc.vector.tensor_tensor(
                scr4[:, 2:3], scr4[:, 2:3], scr4[:, 3:4], op=OP.add
            )
            nc.vector.tensor_tensor(
                scr4[:, 0:1], scr4[:, 0:1], scr4[:, 2:3], op=OP.add
            )
            nc.scalar.activation(
                sclx, scr4[:, 0:1], AF.Sqrt, bias=eps_t, scale=1.0 / DIN
            )
            nc.vector.reciprocal(sclx, sclx)

            # transpose x, apply g, split into f32r hi/lo
            hi_t = hip.tile([P, KT, P], f32r, name="hi")
            lo_t = lop.tile([P, KT, P], f32r, name="lo") if split else None
            for kk in range(KT // 4):
                pt = ptp.tile([P, 4, P], f32, name="pt")
                for q in range(4):
                    k = kk * 4 + q
                    nc.tensor.transpose(
                        pt[:, q, :], x_t[:, k * P : (k + 1) * P], ident32
                    )
                for q in range(4):
                    k = kk * 4 + q
                    nc.scalar.activation(
                        hi_t[:, k, :], pt[:, q, :], AF.Copy,
                        scale=g_sb[:, k : k + 1],
                    )
                    if split:
                        nc.vector.scalar_tensor_tensor(
                            out=lo_t[:, k, :], in0=pt[:, q, :],
                            scalar=g_sb[:, k : k + 1],
                            in1=hi_t[:, k, :].bitcast(f32),
                            op0=OP.mult, op1=OP.subtract,
                        )

            passes = [hi_t, lo_t] if split else [hi_t]
            n_mm = len(passes) * KT

            def do_chunk_mms(pm, c, order_k_outer_pos):
                i = 0
                for lhs in passes:
                    for k in range(KT):
                        nc.tensor.matmul(
                            pm,
                            lhs[:, k, :],
                            w2[:, k, c * CH : (c + 1) * CH],
                            start=(i == 0),
                            stop=(i == n_mm - 1),
                        )
                        i += 1

            def epilogue(pm, c):
                tmp = outp.tile([P, CH], f32, name="ob")
                nc.vector.scalar_tensor_tensor(
                    out=tmp, in0=pm, scalar=sclx,
                    in1=nalpha_b[:, c * CH : (c + 1) * CH],
                    op0=OP.mult, op1=OP.mult,
                )
                ob = outp.tile([P, CH], f32, name="ob")
                nc.vector.tensor_tensor(
                    ob, tmp, bias_b[:, c * CH : (c + 1) * CH], op=OP.add
                )
                nc.sync.dma_start(
                    out=o_d[t * P : (t + 1) * P, c * CH : (c + 1) * CH], in_=ob
                )

            if t == 0:
                # chunk-outer: chunk c only needs w-tiles 4c..4c+3 prepped,
                # so the PE starts before the whole weight prep finishes
                for c in range(NCH):
                    pm = pmm.tile([P, CH], f32, name="pm")
                    do_chunk_mms(pm, c, None)
                    epilogue(pm, c)
            else:
                # k-outer: consecutive matmuls share the stationary operand
                pms = [pmm.tile([P, CH], f32, name="pm") for _ in range(NCH)]
                i = 0
                for lhs in passes:
                    for k in range(KT):
                        for c in range(NCH):
                            nc.tensor.matmul(
                                pms[c],
                                lhs[:, k, :],
                                w2[:, k, c * CH : (c + 1) * CH],
                                start=(i == 0),
                                stop=(i == n_mm - 1),
                            )
                        i += 1
                for c in range(NCH):
                    epilogue(pms[c], c)

    nc.compile()
    return nc


_CACHE = {}


def _get_nc():
    if "nc" not in _CACHE:
        _CACHE["nc"] = build_module()
    return _CACHE["nc"]


def kernel(**inputs) -> np.ndarray:
    nc = _get_nc()
    x = np.ascontiguousarray(
        np.asarray(inputs["x"], dtype=np.float32).reshape(B * S, DIN)
    )
    shards = np.split(x, NCORES, axis=0)
    base = {
        k: np.ascontiguousarray(np.asarray(inputs[k], dtype=np.float32))
        for k in ("weight", "row_scale", "bias", "g")
    }
    in_maps = [{"x": shards[c], **base} for c in range(NCORES)]
    res = bass_utils.run_bass_kernel_spmd(nc, in_maps, list(range(NCORES)))
    out = np.concatenate([res.results[c]["out"] for c in range(NCORES)], axis=0)
    return out.reshape(B, S, DOUT).astype(np.float32)


# revision 21
# speedup vs baseline: 86.6205x; 86.6205x over previous
"""BitLinear (input-RMSNorm + ternary-quantized linear) on 8 TRN2 NeuronCores.

Math (reference):
  xn    = x * rsqrt(mean(x^2, -1) + eps) * g
  w     = weight * rsqrt(mean(weight^2, 1) + eps)          (row RMS norm)
  am    = mean(|w|, 1)
  w_q   = sign(w) * (|w| > 0.5*am)                          (ternary)
  out   = xn @ (w_q * am * row_scale).T + bias

Kernel strategy (per core, data-parallel over B*S rows; weight replicated):
  - The row rsqrt of x commutes with the matmul: apply it to the OUTPUT
    (per-partition scalar).  g is applied to x^T right after the on-chip
    transpose (per-partition in transposed layout).  alpha = am*row_scale
    is applied in the epilogue via a broadcast row (per-free), bias too.
  - The quantized weight is computed on chip as PURE ternary {-1,0,+1}.
    |w| > 0.5*mean|w| is evaluated in the raw-weight domain (the rsqrt
    factor cancels); for the fixed benchmark data the smallest relative
    margin to the threshold is 5.4e-7, far above the ~2e-7 rounding
    differences vs the reference, so no mask flips.
  - Matmul runs on the PE in float32r (11-bit mantissa, 1 cycle/row vs 4
    for fp32).  x is split exactly as x = hi + lo with hi = rnd_f32r(x):
    both halves are f32r-exact, products with ternary weights are exact,
    PSUM accumulates in fp32 -> full fp32 accuracy at half the fp32 cost.
    (SPLIT=False drops the lo pass: 2x faster matmul, ~1e-4 rel err.)
"""

import sys

if "/opt/trn_rl_repo" not in sys.path:
    sys.path.insert(0, "/opt/trn_rl_repo")

from contextlib import ExitStack

import numpy as np

import concourse.bass as bass
import concourse.mybir as mybir
import concourse.tile as tile
from concourse import bacc, bass_utils
from concourse.masks import make_identity

B, S, DIN, DOUT = 4, 4096, 2048, 2048
NCORES = 8
SC = B * S // NCORES      # 2048 rows of x per core
P = 128
KT = DIN // P             # 16 k-tiles
ST = SC // P              # 16 s-tiles per core
CH = 512                  # psum chunk (one bank of fp32)
NCH = DOUT // CH          # 4 chunks
EPS = 1e-8
SPLIT = True              # exact hi/lo split (False: single f32r pass)

f32 = mybir.dt.float32
f32r = mybir.dt.float32r
AF = mybir.ActivationFunctionType
OP = mybir.AluOpType
AX = mybir.AxisListType


def build_module(split=SPLIT, reps=1):
    nc = bacc.Bacc("TRN2", target_bir_lowering=False)
    x_d = nc.declare_dram_parameter("x", [SC, DIN], f32, isOutput=False)
    w_d = nc.declare_dram_parameter("weight", [DOUT, DIN], f32, isOutput=False)
    rs_d = nc.declare_dram_parameter("row_scale", [DOUT, 1], f32, isOutput=False)
    b_d = nc.declare_dram_parameter("bias", [DOUT], f32, isOutput=False)
    g_d = nc.declare_dram_parameter("g", [DIN], f32, isOutput=False)
    o_d = nc.declare_dram_parameter("out", [SC, DOUT], f32, isOutput=True)

    with tile.TileContext(nc) as tc, ExitStack() as ctx:
        const = ctx.enter_context(tc.tile_pool(name="const", bufs=1))
        dramp = ctx.enter_context(tc.tile_pool(name="dramp", bufs=1, space="DRAM"))
        big = ctx.enter_context(tc.tile_pool(name="big", bufs=3))
        hip = ctx.enter_context(tc.tile_pool(name="hip", bufs=2))
        lop = ctx.enter_context(tc.tile_pool(name="lop", bufs=2))
        outp = ctx.enter_context(tc.tile_pool(name="outp", bufs=3))
        smp = ctx.enter_context(tc.tile_pool(name="smp", bufs=4))
        pmm = ctx.enter_context(tc.tile_pool(name="pmm", bufs=5, space="PSUM"))
        ptp = ctx.enter_context(tc.tile_pool(name="ptp", bufs=2, space="PSUM"))

        # ---- constants ----
        w2 = const.tile([P, KT, DOUT], f32r)       # ternary weight, [i, o] layout
        bias_b = const.tile([P, DOUT], f32)        # bias broadcast to all partitions
        nalpha_b = const.tile([P, DOUT], f32)      # -alpha broadcast
        ident32 = const.tile([P, P], f32)
        ident32r = const.tile([P, P], f32r)
        make_identity(nc, ident32)
        nc.vector.tensor_copy(ident32r, ident32)
        eps_t = const.tile([P, 1], f32)
        nc.vector.memset(eps_t, EPS)
        g_sb = const.tile([P, KT], f32)            # g[i], i = k*128 + p  -> [p, k]
        nc.gpsimd.dma_start(out=g_sb, in_=g_d.rearrange("(k p) -> p k", p=P))
        rs_sb = const.tile([P, KT], f32)           # row_scale[o], o = j*128+p
        nc.gpsimd.dma_start(
            out=rs_sb, in_=rs_d.rearrange("(j p) one -> p (j one)", p=P)
        )
        # per-w-tile stats, column j = o-tile j
        sabs = const.tile([P, KT], f32)
        rw = const.tile([P, KT], f32)
        traw = const.tile([P, KT], f32)
        nalpha_c = const.tile([P, KT], f32)
        nalpha_scr = dramp.tile([DOUT], f32)

        # bias broadcast: DRAM [DOUT] replicated over 128 partitions
        bias_ap = b_d[:]
        nc.gpsimd.dma_start(
            out=bias_b,
            in_=bass.AP(
                tensor=bias_ap.tensor, offset=bias_ap.offset,
                ap=[[0, P]] + list(bias_ap.ap),
            ),
        )

        # ---- weight prep: stats -> ternary -> transpose into w2 ----
        def prep_tile(j):
            w_t = big.tile([P, DIN], f32, name="xw")
            nc.sync.dma_start(out=w_t, in_=w_d[j * P : (j + 1) * P, :])
            scr4 = smp.tile([P, 4], f32, name="scr4")

            # ss = sum(w^2) over free dim, dumping squares into spare psum
            for c in range(4):
                dump = pmm.tile([P, CH], f32, name="dump", bufs=1)
                nc.scalar.activation(
                    dump, w_t[:, c * CH : (c + 1) * CH], AF.Square,
                    accum_out=scr4[:, c : c + 1],
                )
            nc.vector.tensor_tensor(
                scr4[:, 0:1], scr4[:, 0:1], scr4[:, 1:2], op=OP.add
            )
            nc.vector.tensor_tensor(
                scr4[:, 2:3], scr4[:, 2:3], scr4[:, 3:4], op=OP.add
            )
            nc.vector.tensor_tensor(
                scr4[:, 0:1], scr4[:, 0:1], scr4[:, 2:3], op=OP.add
            )
            # rw_j = sqrt(ss/DIN + eps) ; then reciprocal in place
            nc.scalar.activation(
                rw[:, j : j + 1], scr4[:, 0:1], AF.Sqrt,
                bias=eps_t, scale=1.0 / DIN,
            )
            nc.vector.reciprocal(rw[:, j : j + 1], rw[:, j : j + 1])
            # sumabs = sum(|w|)
            nc.vector.tensor_reduce(
                sabs[:, j : j + 1], w_t, axis=AX.X, op=OP.add,
                apply_absolute_value=True,
            )
            # threshold in the raw-weight domain: traw = 0.5*mean|w|
            nc.vector.tensor_scalar(
                traw[:, j : j + 1], sabs[:, j : j + 1], 0.5 / DIN, None, op0=OP.mult
            )
            # -alpha = ((-mean|w|) * r) * row_scale
            nc.vector.tensor_scalar(
                scr4[:, 1:2], sabs[:, j : j + 1], -1.0 / DIN, None, op0=OP.mult
            )
            nc.vector.tensor_tensor(
                scr4[:, 1:2], scr4[:, 1:2], rw[:, j : j + 1], op=OP.mult
            )
            nc.vector.tensor_tensor(
                nalpha_c[:, j : j + 1], scr4[:, 1:2], rs_sb[:, j : j + 1],
                op=OP.mult,
            )
            # ntraw into scr4[:,3]
            nc.vector.tensor_scalar(
                scr4[:, 3:4], sabs[:, j : j + 1], -0.5 / DIN, None, op0=OP.mult
            )
            # ternary (negated): e = (w < -traw) - (w > traw)  in {-1,0,+1}
            a_t = lop.tile([P, DIN], f32, name="lo")
            nc.gpsimd.tensor_scalar(
                a_t, w_t, traw[:, j : j + 1], None, op0=OP.is_gt
            )
            e_t = hip.tile([P, DIN], f32r, name="hi")
            nc.vector.scalar_tensor_tensor(
                out=e_t, in0=w_t, scalar=scr4[:, 3:4], in1=a_t,
                op0=OP.is_lt, op1=OP.subtract,
            )
            # transpose e into w2[:, :, j*128:(j+1)*128]
            for kk in range(KT // 4):
                pt = ptp.tile([P, 4, P], f32r, name="pt")
                for q in range(4):
                    k = kk * 4 + q
                    nc.tensor.transpose(
                        pt[:, q, :], e_t[:, k * P : (k + 1) * P], ident32r
                    )
                nc.scalar.activation(
                    w2[:, kk * 4 : kk * 4 + 4, j * P : (j + 1) * P], pt, AF.Copy
                )

        def nalpha_bcast():
            # gather -alpha into o-major DRAM scratch, broadcast to nalpha_b
            nc.gpsimd.dma_start(
                out=nalpha_scr.rearrange("(j p) -> p j", p=P), in_=nalpha_c
            )
            nc.gpsimd.dma_start(
                out=nalpha_b,
                in_=bass.AP(
                    tensor=nalpha_scr.tensor, offset=nalpha_scr.offset,
                    ap=[[0, P]] + list(nalpha_scr.ap),
                ),
            )

        # ---- main loop over s-tiles (front-end pipelined one tile ahead) ----
        def frontend(t):
            """Load x tile t, stats, transpose, apply g, split hi/lo."""
            x_t = big.tile([P, DIN], f32, name="xw")
            nc.sync.dma_start(out=x_t, in_=x_d[t * P : (t + 1) * P, :])
            scr4 = smp.tile([P, 4], f32, name="scr4")
            for c in range(4):
                dump = pmm.tile([P, CH], f32, name="dump", bufs=1)
                nc.scalar.activation(
                    dump, x_t[:, c * CH : (c + 1) * CH], AF.Square,
                    accum_out=scr4[:, c : c + 1],
                )
            sclx = smp.tile([P, 1], f32, name="sclx")
            nc.vector.tensor_tensor(
                scr4[:, 0:1], scr4[:, 0:1], scr4[:, 1:2], op=OP.add
            )
            nc.vector.tensor_tensor(
                scr4[:, 2:3], scr4[:, 2:3], scr4[:, 3:4], op=OP.add
            )
            nc.vector.tensor_tensor(
                scr4[:, 0:1], scr4[:, 0:1], scr4[:, 2:3], op=OP.add
            )
            nc.scalar.activation(
                sclx, scr4[:, 0:1], AF.Sqrt, bias=eps_t, scale=1.0 / DIN
            )
            nc.vector.reciprocal(sclx, sclx)

            hi_t = hip.tile([P, KT, P], f32r, name="hi")
            lo_t = lop.tile([P, KT, P], f32r, name="lo") if split else None
            for kk in range(KT // 4):
                pt = ptp.tile([P, 4, P], f32, name="pt")
                for q in range(4):
                    k = kk * 4 + q
                    nc.tensor.transpose(
                        pt[:, q, :], x_t[:, k * P : (k + 1) * P], ident32
                    )
                for q in range(4):
                    k = kk * 4 + q
                    nc.scalar.activation(
                        hi_t[:, k, :], pt[:, q, :], AF.Copy,
                        scale=g_sb[:, k : k + 1],
                    )
                    if split:
                        nc.vector.scalar_tensor_tensor(
                            out=lo_t[:, k, :], in0=pt[:, q, :],
                            scalar=g_sb[:, k : k + 1],
                            in1=hi_t[:, k, :].bitcast(f32),
                            op0=OP.mult, op1=OP.subtract,
                        )
            return hi_t, lo_t, sclx

        def backend(t, fr):
            hi_t, lo_t, sclx = fr
            passes = [hi_t, lo_t] if split else [hi_t]
            n_mm = len(passes) * KT

            def epilogue(pm, c):
                tmp = outp.tile([P, CH], f32, name="ob")
                nc.vector.scalar_tensor_tensor(
                    out=tmp, in0=pm, scalar=sclx,
                    in1=nalpha_b[:, c * CH : (c + 1) * CH],
                    op0=OP.mult, op1=OP.mult,
                )
                ob = outp.tile([P, CH], f32, name="ob")
                # bias add on the otherwise-idle gpsimd engine
                nc.gpsimd.tensor_tensor(
                    ob, tmp, bias_b[:, c * CH : (c + 1) * CH], op=OP.add
                )
                nc.sync.dma_start(
                    out=o_d[t * P : (t + 1) * P, c * CH : (c + 1) * CH], in_=ob
                )

            if t < 4:
                # chunk-outer: chunk c only needs w-tiles 4c..4c+3 prepped,
                # so the PE starts before the whole weight prep finishes
                for c in range(NCH):
                    pm = pmm.tile([P, CH], f32, name="pm")
                    i = 0
                    for lhs in passes:
                        for k in range(KT):
                            nc.tensor.matmul(
                                pm,
                                lhs[:, k, :],
                                w2[:, k, c * CH : (c + 1) * CH],
                                start=(i == 0),
                                stop=(i == n_mm - 1),
                            )
                            i += 1
                    epilogue(pm, c)
            else:
                # k-outer: consecutive matmuls share the stationary operand
                pms = [pmm.tile([P, CH], f32, name="pm") for _ in range(NCH)]
                i = 0
                for lhs in passes:
                    for k in range(KT):
                        for c in range(NCH):
                            nc.tensor.matmul(
                                pms[c],
                                lhs[:, k, :],
                                w2[:, k, c * CH : (c + 1) * CH],
                                start=(i == 0),
                                stop=(i == n_mm - 1),
                            )
                        i += 1
                for c in range(NCH):
                    epilogue(pms[c], c)

        for rep in range(reps):
            # frontend(0) first: its x DMA and transposes run during prep,
            # and backend(0) (chunk-gated) fills the PE while prep finishes
            fr = frontend(0)
            for j in range(KT):
                prep_tile(j)
            nalpha_bcast()
            for t in range(ST):
                backend(t, fr)
                fr = frontend(t + 1) if t + 1 < ST else None

    nc.compile()
    return nc


_CACHE = {}


def _get_nc():
    if "nc" not in _CACHE:
        _CACHE["nc"] = build_module()
    return _CACHE["nc"]


def kernel(**inputs) -> np.ndarray:
    nc = _get_nc()
    x = np.ascontiguousarray(
        np.asarray(inputs["x"], dtype=np.float32).reshape(B * S, DIN)
    )
    shards = np.split(x, NCORES, axis=0)
    base = {
        k: np.ascontiguousarray(np.asarray(inputs[k], dtype=np.float32))
        for k in ("weight", "row_scale", "bias", "g")
    }
    in_maps = [{"x": shards[c], **base} for c in range(NCORES)]
    res = bass_utils.run_bass_kernel_spmd(nc, in_maps, list(range(NCORES)))
    out = np.concatenate([res.results[c]["out"] for c in range(NCORES)], axis=0)
    return out.reshape(B, S, DOUT).astype(np.float32)


# revision 22
# speedup vs baseline: 89.7909x; 1.0366x over previous
"""BitLinear (input-RMSNorm + ternary-quantized linear) on 8 TRN2 NeuronCores.

Math (reference):
  xn    = x * rsqrt(mean(x^2, -1) + eps) * g
  w     = weight * rsqrt(mean(weight^2, 1) + eps)          (row RMS norm)
  am    = mean(|w|, 1)
  w_q   = sign(w) * (|w| > 0.5*am)                          (ternary)
  out   = xn @ (w_q * am * row_scale).T + bias

Kernel strategy (per core, data-parallel over B*S rows; weight replicated):
  - The row rsqrt of x commutes with the matmul: apply it to the OUTPUT
    (per-partition scalar).  g is applied to x^T right after the on-chip
    transpose (per-partition in transposed layout).  alpha = am*row_scale
    is applied in the epilogue via a broadcast row (per-free), bias too.
  - The quantized weight is computed on chip as PURE ternary {-1,0,+1}.
    |w| > 0.5*mean|w| is evaluated in the raw-weight domain (the rsqrt
    factor cancels); for the fixed benchmark data the smallest relative
    margin to the threshold is 5.4e-7, far above the ~2e-7 rounding
    differences vs the reference, so no mask flips.
  - Matmul runs on the PE in float32r (11-bit mantissa, 1 cycle/row vs 4
    for fp32).  x is split exactly as x = hi + lo with hi = rnd_f32r(x):
    both halves are f32r-exact, products with ternary weights are exact,
    PSUM accumulates in fp32 -> full fp32 accuracy at half the fp32 cost.
    (SPLIT=False drops the lo pass: 2x faster matmul, ~1e-4 rel err.)
"""

import sys

try:
    import concourse.bass  # noqa: F401
except ImportError:
    for _p in ("/opt/trn_rl_repo", "/root/.axon_site/_ro/trn_rl_repo"):
        if _p not in sys.path:
            sys.path.insert(0, _p)

from contextlib import ExitStack

import numpy as np

import concourse.bass as bass
import concourse.mybir as mybir
import concourse.tile as tile
from concourse import bacc, bass_utils
from concourse.masks import make_identity

B, S, DIN, DOUT = 4, 4096, 2048, 2048
NCORES = 8
SC = B * S // NCORES      # 2048 rows of x per core
P = 128
KT = DIN // P             # 16 k-tiles
ST = SC // P              # 16 s-tiles per core
CH = 512                  # psum chunk (one bank of fp32)
NCH = DOUT // CH          # 4 chunks
EPS = 1e-8
SPLIT = True              # exact hi/lo split (False: single f32r pass)

f32 = mybir.dt.float32
f32r = mybir.dt.float32r
AF = mybir.ActivationFunctionType
OP = mybir.AluOpType
AX = mybir.AxisListType


def build_module(split=SPLIT, reps=1):
    nc = bacc.Bacc("TRN2", target_bir_lowering=False)
    x_d = nc.declare_dram_parameter("x", [SC, DIN], f32, isOutput=False)
    w_d = nc.declare_dram_parameter("weight", [DOUT, DIN], f32, isOutput=False)
    rs_d = nc.declare_dram_parameter("row_scale", [DOUT, 1], f32, isOutput=False)
    b_d = nc.declare_dram_parameter("bias", [DOUT], f32, isOutput=False)
    g_d = nc.declare_dram_parameter("g", [DIN], f32, isOutput=False)
    o_d = nc.declare_dram_parameter("out", [SC, DOUT], f32, isOutput=True)

    with tile.TileContext(nc) as tc, ExitStack() as ctx:
        const = ctx.enter_context(tc.tile_pool(name="const", bufs=1))
        dramp = ctx.enter_context(tc.tile_pool(name="dramp", bufs=1, space="DRAM"))
        big = ctx.enter_context(tc.tile_pool(name="big", bufs=3))
        hip = ctx.enter_context(tc.tile_pool(name="hip", bufs=2))
        lop = ctx.enter_context(tc.tile_pool(name="lop", bufs=2))
        outp = ctx.enter_context(tc.tile_pool(name="outp", bufs=3))
        smp = ctx.enter_context(tc.tile_pool(name="smp", bufs=4))
        pmm = ctx.enter_context(tc.tile_pool(name="pmm", bufs=5, space="PSUM"))
        ptp = ctx.enter_context(tc.tile_pool(name="ptp", bufs=2, space="PSUM"))

        # ---- constants ----
        w2 = const.tile([P, KT, DOUT], f32r)       # ternary weight, [i, o] layout
        bias_b = const.tile([P, DOUT], f32)        # bias broadcast to all partitions
        nalpha_b = const.tile([P, DOUT], f32)      # -alpha broadcast
        ident32 = const.tile([P, P], f32)
        ident32r = const.tile([P, P], f32r)
        make_identity(nc, ident32)
        nc.vector.tensor_copy(ident32r, ident32)
        eps_t = const.tile([P, 1], f32)
        nc.vector.memset(eps_t, EPS)
        g_sb = const.tile([P, KT], f32)            # g[i], i = k*128 + p  -> [p, k]
        nc.gpsimd.dma_start(out=g_sb, in_=g_d.rearrange("(k p) -> p k", p=P))
        rs_sb = const.tile([P, KT], f32)           # row_scale[o], o = j*128+p
        nc.gpsimd.dma_start(
            out=rs_sb, in_=rs_d.rearrange("(j p) one -> p (j one)", p=P)
        )
        # per-w-tile stats, column j = o-tile j
        sabs = const.tile([P, KT], f32)
        rw = const.tile([P, KT], f32)
        traw = const.tile([P, KT], f32)
        nalpha_c = const.tile([P, KT], f32)
        nalpha_scr = dramp.tile([DOUT], f32)

        # bias broadcast: DRAM [DOUT] replicated over 128 partitions
        bias_ap = b_d[:]
        nc.gpsimd.dma_start(
            out=bias_b,
            in_=bass.AP(
                tensor=bias_ap.tensor, offset=bias_ap.offset,
                ap=[[0, P]] + list(bias_ap.ap),
            ),
        )

        # ---- weight prep: stats -> ternary -> transpose into w2 ----
        def prep_tile(j):
            w_t = big.tile([P, DIN], f32, name="xw")
            nc.sync.dma_start(out=w_t, in_=w_d[j * P : (j + 1) * P, :])
            scr4 = smp.tile([P, 4], f32, name="scr4")

            # ss = sum(w^2) over free dim, dumping squares into spare psum
            for c in range(4):
                dump = pmm.tile([P, CH], f32, name="dump", bufs=1)
                nc.scalar.activation(
                    dump, w_t[:, c * CH : (c + 1) * CH], AF.Square,
                    accum_out=scr4[:, c : c + 1],
                )
            nc.vector.tensor_tensor(
                scr4[:, 0:1], scr4[:, 0:1], scr4[:, 1:2], op=OP.add
            )
            nc.vector.tensor_tensor(
                scr4[:, 2:3], scr4[:, 2:3], scr4[:, 3:4], op=OP.add
            )
            nc.vector.tensor_tensor(
                scr4[:, 0:1], scr4[:, 0:1], scr4[:, 2:3], op=OP.add
            )
            # rw_j = sqrt(ss/DIN + eps) ; then reciprocal in place
            nc.scalar.activation(
                rw[:, j : j + 1], scr4[:, 0:1], AF.Sqrt,
                bias=eps_t, scale=1.0 / DIN,
            )
            nc.vector.reciprocal(rw[:, j : j + 1], rw[:, j : j + 1])
            # sumabs = sum(|w|)
            nc.vector.tensor_reduce(
                sabs[:, j : j + 1], w_t, axis=AX.X, op=OP.add,
                apply_absolute_value=True,
            )
            # threshold in the raw-weight domain: traw = 0.5*mean|w|
            nc.vector.tensor_scalar(
                traw[:, j : j + 1], sabs[:, j : j + 1], 0.5 / DIN, None, op0=OP.mult
            )
            # -alpha = ((-mean|w|) * r) * row_scale
            nc.vector.tensor_scalar(
                scr4[:, 1:2], sabs[:, j : j + 1], -1.0 / DIN, None, op0=OP.mult
            )
            nc.vector.tensor_tensor(
                scr4[:, 1:2], scr4[:, 1:2], rw[:, j : j + 1], op=OP.mult
            )
            nc.vector.tensor_tensor(
                nalpha_c[:, j : j + 1], scr4[:, 1:2], rs_sb[:, j : j + 1],
                op=OP.mult,
            )
            # ntraw into scr4[:,3]
            nc.vector.tensor_scalar(
                scr4[:, 3:4], sabs[:, j : j + 1], -0.5 / DIN, None, op0=OP.mult
            )
            # ternary (negated): e = (w < -traw) - (w > traw)  in {-1,0,+1}
            a_t = lop.tile([P, DIN], f32, name="lo")
            nc.gpsimd.tensor_scalar(
                a_t, w_t, traw[:, j : j + 1], None, op0=OP.is_gt
            )
            e_t = hip.tile([P, DIN], f32r, name="hi")
            nc.vector.scalar_tensor_tensor(
                out=e_t, in0=w_t, scalar=scr4[:, 3:4], in1=a_t,
                op0=OP.is_lt, op1=OP.subtract,
            )
            # transpose e into w2[:, :, j*128:(j+1)*128]
            for kk in range(KT // 4):
                pt = ptp.tile([P, 4, P], f32r, name="pt")
                for q in range(4):
                    k = kk * 4 + q
                    nc.tensor.transpose(
                        pt[:, q, :], e_t[:, k * P : (k + 1) * P], ident32r
                    )
                nc.scalar.activation(
                    w2[:, kk * 4 : kk * 4 + 4, j * P : (j + 1) * P], pt, AF.Copy
                )

        def nalpha_bcast():
            # gather -alpha into o-major DRAM scratch, broadcast to nalpha_b
            nc.gpsimd.dma_start(
                out=nalpha_scr.rearrange("(j p) -> p j", p=P), in_=nalpha_c
            )
            nc.gpsimd.dma_start(
                out=nalpha_b,
                in_=bass.AP(
                    tensor=nalpha_scr.tensor, offset=nalpha_scr.offset,
                    ap=[[0, P]] + list(nalpha_scr.ap),
                ),
            )

        # ---- main loop over s-tiles (front-end pipelined one tile ahead) ----
        def frontend(t):
            """Load x tile t, stats, transpose, apply g, split hi/lo."""
            x_t = big.tile([P, DIN], f32, name="xw")
            nc.sync.dma_start(out=x_t, in_=x_d[t * P : (t + 1) * P, :])
            scr4 = smp.tile([P, 4], f32, name="scr4")
            for c in range(4):
                dump = pmm.tile([P, CH], f32, name="dump", bufs=1)
                nc.scalar.activation(
                    dump, x_t[:, c * CH : (c + 1) * CH], AF.Square,
                    accum_out=scr4[:, c : c + 1],
                )
            sclx = smp.tile([P, 1], f32, name="sclx")
            nc.vector.tensor_tensor(
                scr4[:, 0:1], scr4[:, 0:1], scr4[:, 1:2], op=OP.add
            )
            nc.vector.tensor_tensor(
                scr4[:, 2:3], scr4[:, 2:3], scr4[:, 3:4], op=OP.add
            )
            nc.vector.tensor_tensor(
                scr4[:, 0:1], scr4[:, 0:1], scr4[:, 2:3], op=OP.add
            )
            nc.scalar.activation(
                sclx, scr4[:, 0:1], AF.Sqrt, bias=eps_t, scale=1.0 / DIN
            )
            nc.vector.reciprocal(sclx, sclx)

            hi_t = hip.tile([P, KT, P], f32r, name="hi")
            lo_t = lop.tile([P, KT, P], f32r, name="lo") if split else None
            for kk in range(KT // 4):
                pt = ptp.tile([P, 4, P], f32, name="pt")
                for q in range(4):
                    k = kk * 4 + q
                    nc.tensor.transpose(
                        pt[:, q, :], x_t[:, k * P : (k + 1) * P], ident32
                    )
                for q in range(4):
                    k = kk * 4 + q
                    nc.scalar.activation(
                        hi_t[:, k, :], pt[:, q, :], AF.Copy,
                        scale=g_sb[:, k : k + 1],
                    )
                    if split:
                        nc.vector.scalar_tensor_tensor(
                            out=lo_t[:, k, :], in0=pt[:, q, :],
                            scalar=g_sb[:, k : k + 1],
                            in1=hi_t[:, k, :].bitcast(f32),
                            op0=OP.mult, op1=OP.subtract,
                        )
            return hi_t, lo_t, sclx

        def backend(t, fr):
            hi_t, lo_t, sclx = fr
            passes = [hi_t, lo_t] if split else [hi_t]
            n_mm = len(passes) * KT

            def epilogue(pm, c):
                tmp = outp.tile([P, CH], f32, name="ob")
                nc.vector.scalar_tensor_tensor(
                    out=tmp, in0=pm, scalar=sclx,
                    in1=nalpha_b[:, c * CH : (c + 1) * CH],
                    op0=OP.mult, op1=OP.mult,
                )
                ob = outp.tile([P, CH], f32, name="ob")
                # bias add on the otherwise-idle gpsimd engine
                nc.gpsimd.tensor_tensor(
                    ob, tmp, bias_b[:, c * CH : (c + 1) * CH], op=OP.add
                )
                nc.sync.dma_start(
                    out=o_d[t * P : (t + 1) * P, c * CH : (c + 1) * CH], in_=ob
                )

            if t < 4:
                # chunk-outer: chunk c only needs w-tiles 4c..4c+3 prepped,
                # so the PE starts before the whole weight prep finishes
                for c in range(NCH):
                    pm = pmm.tile([P, CH], f32, name="pm")
                    i = 0
                    for lhs in passes:
                        for k in range(KT):
                            nc.tensor.matmul(
                                pm,
                                lhs[:, k, :],
                                w2[:, k, c * CH : (c + 1) * CH],
                                start=(i == 0),
                                stop=(i == n_mm - 1),
                            )
                            i += 1
                    epilogue(pm, c)
            else:
                # k-outer: consecutive matmuls share the stationary operand
                pms = [pmm.tile([P, CH], f32, name="pm") for _ in range(NCH)]
                i = 0
                for lhs in passes:
                    for k in range(KT):
                        for c in range(NCH):
                            nc.tensor.matmul(
                                pms[c],
                                lhs[:, k, :],
                                w2[:, k, c * CH : (c + 1) * CH],
                                start=(i == 0),
                                stop=(i == n_mm - 1),
                            )
                        i += 1
                for c in range(NCH):
                    epilogue(pms[c], c)

        for rep in range(reps):
            # frontend(0) first: its x DMA and transposes run during prep,
            # and backend(0) (chunk-gated) fills the PE while prep finishes
            fr = frontend(0)
            for j in range(KT):
                prep_tile(j)
            nalpha_bcast()
            for t in range(ST):
                backend(t, fr)
                fr = frontend(t + 1) if t + 1 < ST else None

    nc.compile()
    return nc


_CACHE = {}


def _get_nc():
    if "nc" not in _CACHE:
        _CACHE["nc"] = build_module()
    return _CACHE["nc"]


def kernel(**inputs) -> np.ndarray:
    nc = _get_nc()
    x = np.ascontiguousarray(
        np.asarray(inputs["x"], dtype=np.float32).reshape(B * S, DIN)
    )
    shards = np.split(x, NCORES, axis=0)
    base = {
        k: np.ascontiguousarray(np.asarray(inputs[k], dtype=np.float32))
        for k in ("weight", "row_scale", "bias", "g")
    }
    in_maps = [{"x": shards[c], **base} for c in range(NCORES)]
    res = bass_utils.run_bass_kernel_spmd(nc, in_maps, list(range(NCORES)))
    out = np.concatenate([res.results[c]["out"] for c in range(NCORES)], axis=0)
    return out.reshape(B, S, DOUT).astype(np.float32)
